# revision 3
# baseline (speedup 1.0000x reference)
"""Trainium2 Bass kernel for nn_CirculantSTRING.

Math: out[b,n,:] = irfft(exp(i*theta(n,:)) * rfft(x[b,n,:]), n=D)
where theta(n,f) = 2*(p0[n]*Im(rfft(circ0))[f] + p1[n]*Im(rfft(circ1))[f]).

Implementation per core (data-parallel over batch, 4 batches/core):
  - Forward rfft as a real matmul with packed output
    fi = [R_0..R_383 | R_384, I_1..I_383] (768 wide).
  - Phase rotation tables cos/sin(theta) computed on device from circ +
    positions (small matmuls + range-reduced Sin activation).
  - Elementwise rotation in (f partitions, rows free) layout.
  - Inverse rfft as a second real matmul back to (rows, d) layout.
Matmuls run in float32r (1 cycle/row on the PE at N>=256; ~11 mantissa
bits), phase tables and rotation in fp32.
"""
import math
from contextlib import ExitStack

import numpy as np

import concourse.bacc as bacc
import concourse.tile as tile
from concourse import mybir
from concourse import bass_utils
from concourse.masks import make_identity

F32 = mybir.dt.float32
F32R = mybir.dt.float32r
I32 = mybir.dt.int32

B, N, D = 32, 1024, 768
NCORES = 8
BS = B // NCORES          # batches per core
P = 128                   # partitions
NF = D // 2 + 1           # 385 rfft freqs
NCH = D // P              # 6 chunks of 128 along d / fi
ROWTILE = 512             # rows processed per main-loop tile (one b, half of N)
NG = ROWTILE // P         # 4 row groups per tile

TWOPI = 2.0 * math.pi


def _dft_matrices():
    """Forward FP (d, fi) and inverse GP (fi, d) packed real-DFT matrices.

    fi packing: [R_0..R_383 | R_384, I_1..I_383].
    """
    d = np.arange(D)
    f = np.arange(NF)
    ang = 2.0 * np.pi * np.outer(d, f) / D      # (768, 385)
    C = np.cos(ang)
    S = np.sin(ang)
    FP = np.zeros((D, D), np.float32)
    FP[:, 0:384] = C[:, 0:384]
    FP[:, 384] = C[:, 384]                      # R_384 = sum x_d cos(pi d)
    FP[:, 385:768] = -S[:, 1:384]               # I_f = -sum x_d sin(...)
    w = np.full(NF, 2.0)
    w[0] = 1.0
    w[384] = 1.0
    GP = np.zeros((D, D), np.float32)
    GP[0:384, :] = (w[0:384, None] * C[:, 0:384].T) / D
    GP[384, :] = C[:, 384].T / D                # R_384 row
    GP[385:768, :] = -(2.0 * S[:, 1:384].T) / D  # I_f rows
    return FP, GP


def build_kernel(mm_dtype=F32R):
    nc = bacc.Bacc("TRN2", target_bir_lowering=False, debug=False,
                   num_devices=NCORES)
    x = nc.dram_tensor("x", [BS, N, D], F32, kind="ExternalInput").ap()
    circ = nc.dram_tensor("circ", [2, D], F32, kind="ExternalInput").ap()
    positions = nc.dram_tensor("positions", [N, 2], I32,
                               kind="ExternalInput").ap()
    fp_c = nc.dram_tensor("fp_c", [D, D], F32, kind="ExternalInput").ap()
    gp_c = nc.dram_tensor("gp_c", [D, D], F32, kind="ExternalInput").ap()
    out = nc.dram_tensor("out", [BS, N, D], F32, kind="ExternalOutput").ap()

    with tile.TileContext(nc) as tc, ExitStack() as ctx:
        consts = ctx.enter_context(tc.tile_pool(name="consts", bufs=1))
        stage = ctx.enter_context(tc.tile_pool(name="stage", bufs=1))
        tabs = ctx.enter_context(tc.tile_pool(name="tabs", bufs=1))
        xio = ctx.enter_context(tc.tile_pool(name="xio", bufs=2))
        work = ctx.enter_context(tc.tile_pool(name="work", bufs=2))

        ident = consts.tile([P, P], F32, tag="ident")
        make_identity(nc, ident)

        # ---- load + round DFT matrices to fp32r ----
        FPt, GPt = [], []
        for name, src in (("fp", fp_c), ("gp", gp_c)):
            for c in range(NCH):
                st = stage.tile([P, D], F32, tag="mstage")
                nc.sync.dma_start(out=st, in_=src[c * P:(c + 1) * P, :])
                t = consts.tile([P, D], mm_dtype, tag=f"{name}{c}")
                nc.scalar.copy(out=t, in_=st)
                (FPt if name == "fp" else GPt).append(t)

        # ---- phase tables ----
        # posT (2, N) int32 -> *2.0 f32
        posT = tabs.tile([2, N], I32, tag="posT")
        nc.sync.dma_start(out=posT, in_=positions.rearrange("n k -> k n"))
        posTf = tabs.tile([2, N], F32, tag="posTf")
        nc.vector.tensor_scalar_mul(posTf, posT, 2.0)

        # circT (d on partitions): 6 chunks of (128, 2)
        circT = tabs.tile([P, 2 * NCH], F32, tag="circT")
        cr = circ.rearrange("k d -> d k")
        for c in range(NCH):
            nc.sync.dma_start(out=circT[:, 2 * c:2 * c + 2],
                              in_=cr[c * P:(c + 1) * P, :])

        # s2[k, f] = Im(rfft(circ_k))[f] for f=1..383 via FP imag columns
        ps0 = tc.tile_pool(name="ps0", bufs=1, space="PSUM")
        psum = ps0.__enter__()
        s2ps = psum.tile([2, 384], F32, tag="s2ps")
        for c in range(NCH):
            nc.tensor.matmul(s2ps[:, 1:384], circT[:, 2 * c:2 * c + 2],
                             FPt[c][:, 385:768].bitcast(F32),
                             start=(c == 0), stop=(c == NCH - 1))
        s2 = tabs.tile([2, 384], F32, tag="s2")
        nc.vector.memset(s2[:, 0:1], 0.0)
        nc.vector.tensor_copy(out=s2[:, 1:384], in_=s2ps[:, 1:384])

        # theta chunks (128 f, N) = s2_chunk^T @ posTf ; then sin/cos tables
        cT, sT = [], []
        for j in range(3):
            thps = psum.tile([P, N], F32, tag="thps")
            for h in range(2):
                nc.tensor.matmul(thps[:, h * 512:(h + 1) * 512],
                                 s2[:, j * P:(j + 1) * P],
                                 posTf[:, h * 512:(h + 1) * 512],
                                 start=True, stop=True)
            sj = tabs.tile([P, N], F32, tag=f"sT{j}")
            cj = tabs.tile([P, N], F32, tag=f"cT{j}")
            # sin: r = round(th/2pi); sin(th - 2pi r)
            t1 = stage.tile([P, N], F32, tag="pt")
            r1 = stage.tile([P, N], I32, tag="pr")
            u1 = stage.tile([P, N], F32, tag="pu")
            red = stage.tile([P, N], F32, tag="pred")
            nc.vector.tensor_scalar_mul(t1, thps, 1.0 / TWOPI)
            nc.vector.tensor_copy(out=r1, in_=t1)
            nc.vector.tensor_scalar_mul(u1, r1, -TWOPI)
            nc.vector.tensor_add(red, thps, u1)
            nc.scalar.activation(out=sj, in_=red,
                                 func=mybir.ActivationFunctionType.Sin)
            # cos: r2 = round(th/2pi + 1/4); cos = sin(th - 2pi r2 + pi/2)
            t2 = stage.tile([P, N], F32, tag="pt")
            r2 = stage.tile([P, N], I32, tag="pr")
            u2 = stage.tile([P, N], F32, tag="pu")
            red2 = stage.tile([P, N], F32, tag="pred")
            nc.vector.tensor_scalar(t2, thps, 1.0 / TWOPI, 0.25,
                                    op0=mybir.AluOpType.mult,
                                    op1=mybir.AluOpType.add)
            nc.vector.tensor_copy(out=r2, in_=t2)
            nc.vector.tensor_scalar(u2, r2, -TWOPI, math.pi / 2,
                                    op0=mybir.AluOpType.mult,
                                    op1=mybir.AluOpType.add)
            nc.vector.tensor_add(red2, thps, u2)
            nc.scalar.activation(out=cj, in_=red2,
                                 func=mybir.ActivationFunctionType.Sin)
            sT.append(sj)
            cT.append(cj)
        ps0.__exit__(None, None, None)

        # ---- main loop ----
        pst_pool = ctx.enter_context(tc.tile_pool(name="pst", bufs=2, space="PSUM"))
        psf = ctx.enter_context(tc.tile_pool(name="psf", bufs=3, space="PSUM"))
        psi = ctx.enter_context(tc.tile_pool(name="psi", bufs=1, space="PSUM"))
        for b in range(BS):
            for h in range(2):
                n0 = h * ROWTILE
                # load 4 row groups
                xg = []
                for g in range(NG):
                    t = xio.tile([P, D], F32, tag=f"x{g}")
                    nc.sync.dma_start(
                        out=t, in_=x[b, n0 + g * P:n0 + (g + 1) * P, :])
                    xg.append(t)
                # transpose to (d, rows) as 6 chunks of (128, 512), fp32r
                XT = []
                for c in range(NCH):
                    pst = pst_pool.tile([P, ROWTILE], F32, tag="pst")
                    for g in range(NG):
                        nc.tensor.transpose(pst[:, g * P:(g + 1) * P],
                                            xg[g][:, c * P:(c + 1) * P],
                                            ident)
                    xt = work.tile([P, ROWTILE], mm_dtype, tag=f"xt{c}")
                    nc.scalar.copy(out=xt, in_=pst)
                    XT.append(xt)
                # forward + rotation, pair (j, 3+j)
                RI = [None] * NCH
                for j in range(3):
                    pR = psf.tile([P, ROWTILE], F32, tag="psf")
                    pI = psf.tile([P, ROWTILE], F32, tag="psf")
                    for c in range(NCH):
                        nc.tensor.matmul(pR, FPt[c][:, j * P:(j + 1) * P],
                                         XT[c], start=(c == 0),
                                         stop=(c == NCH - 1))
                    for c in range(NCH):
                        nc.tensor.matmul(pI,
                                         FPt[c][:, (3 + j) * P:(4 + j) * P],
                                         XT[c], start=(c == 0),
                                         stop=(c == NCH - 1))
                    cs = cT[j][:, n0:n0 + ROWTILE]
                    sn = sT[j][:, n0:n0 + ROWTILE]
                    t1 = work.tile([P, ROWTILE], F32, tag="rta")
                    t2 = work.tile([P, ROWTILE], F32, tag="rtb")
                    nc.vector.tensor_mul(t1, pR, cs)
                    nc.vector.tensor_mul(t2, pI, sn)
                    rp = work.tile([P, ROWTILE], mm_dtype, tag=f"ri{j}")
                    ip = work.tile([P, ROWTILE], mm_dtype, tag=f"ri{3 + j}")
                    nc.gpsimd.tensor_sub(rp, t1, t2)
                    t3 = work.tile([P, ROWTILE], F32, tag="rta")
                    t4 = work.tile([P, ROWTILE], F32, tag="rtb")
                    nc.vector.tensor_mul(t3, pR, sn)
                    nc.vector.tensor_mul(t4, pI, cs)
                    nc.gpsimd.tensor_add(ip, t3, t4)
                    RI[j] = rp
                    RI[3 + j] = ip
                # inverse, per row group
                for g in range(NG):
                    pa = psi.tile([P, 512], F32, tag="pa")
                    pb = psi.tile([P, 256], F32, tag="pb")
                    for c in range(NCH):
                        lhs = RI[c][:, g * P:(g + 1) * P]
                        nc.tensor.matmul(pa, lhs, GPt[c][:, 0:512],
                                         start=(c == 0), stop=(c == NCH - 1))
                        nc.tensor.matmul(pb, lhs, GPt[c][:, 512:768],
                                         start=(c == 0), stop=(c == NCH - 1))
                    osb = xio.tile([P, D], F32, tag=f"o{g % 2}")
                    if g % 2 == 0:
                        nc.scalar.copy(out=osb[:, 0:512], in_=pa)
                        nc.scalar.copy(out=osb[:, 512:768], in_=pb)
                    else:
                        nc.vector.tensor_copy(out=osb[:, 0:512], in_=pa)
                        nc.vector.tensor_copy(out=osb[:, 512:768], in_=pb)
                    nc.sync.dma_start(
                        out=out[b, n0 + g * P:n0 + (g + 1) * P, :], in_=osb)
    nc.finalize()
    return nc


_NC_CACHE = {}


def kernel(x, circ, positions):
    x = np.ascontiguousarray(x, dtype=np.float32)
    circ = np.ascontiguousarray(circ, dtype=np.float32)
    positions = np.ascontiguousarray(positions, dtype=np.int32)
    if "nc" not in _NC_CACHE:
        _NC_CACHE["nc"] = build_kernel()
    nc = _NC_CACHE["nc"]
    FP, GP = _dft_matrices()
    in_maps = []
    for core in range(NCORES):
        in_maps.append({
            "x": x[core * BS:(core + 1) * BS],
            "circ": circ,
            "positions": positions,
            "fp_c": FP,
            "gp_c": GP,
        })
    res = bass_utils.run_bass_kernel_spmd(nc, in_maps,
                                          core_ids=list(range(NCORES)))
    out = np.concatenate([res.results[c]["out"] for c in range(NCORES)],
                         axis=0)
    return out


if __name__ == "__main__":
    rng = np.random.default_rng(0)
    x = rng.standard_normal((B, N, D)).astype(np.float32)
    circ = (rng.standard_normal((2, D)) * 0.01).astype(np.float32)
    positions = rng.integers(0, 32, (N, 2)).astype(np.int32)
    out = kernel(x=x, circ=circ, positions=positions)
    print("out", out.shape, out.dtype)


# revision 4
# speedup vs baseline: 17.2971x; 17.2971x over previous
"""Trainium2 Bass kernel for nn_CirculantSTRING.

Math: out[b,n,:] = irfft(exp(i*theta(n,:)) * rfft(x[b,n,:]), n=D)
where theta(n,f) = 2*(p0[n]*Im(rfft(circ0))[f] + p1[n]*Im(rfft(circ1))[f]).

Implementation per core (data-parallel over batch, 4 batches/core):
  - Forward rfft as a real matmul with packed output
    fi = [R_0..R_383 | R_384, I_1..I_383] (768 wide).
  - Phase rotation tables cos/sin(theta) computed on device from circ +
    positions (small matmuls + range-reduced Sin activation).
  - Elementwise rotation in (f partitions, rows free) layout.
  - Inverse rfft as a second real matmul back to (rows, d) layout.
Matmuls run in float32r (1 cycle/row on the PE at N>=256; ~11 mantissa
bits), phase tables and rotation in fp32.
"""
import math
from contextlib import ExitStack

import numpy as np

import concourse.bacc as bacc
import concourse.tile as tile
from concourse import mybir
from concourse import bass_utils
from concourse.masks import make_identity

F32 = mybir.dt.float32
F32R = mybir.dt.float32r
I32 = mybir.dt.int32

B, N, D = 32, 1024, 768
NCORES = 8
BS = B // NCORES          # batches per core
P = 128                   # partitions
NF = D // 2 + 1           # 385 rfft freqs
NCH = D // P              # 6 chunks of 128 along d / fi
ROWTILE = 512             # rows processed per main-loop tile (one b, half of N)
NG = ROWTILE // P         # 4 row groups per tile

TWOPI = 2.0 * math.pi


def _dft_matrices():
    """Forward FP (d, fi) and inverse GP (fi, d) packed real-DFT matrices.

    fi packing: [R_0..R_383 | R_384, I_1..I_383].
    """
    d = np.arange(D)
    f = np.arange(NF)
    ang = 2.0 * np.pi * np.outer(d, f) / D      # (768, 385)
    C = np.cos(ang)
    S = np.sin(ang)
    FP = np.zeros((D, D), np.float32)
    FP[:, 0:384] = C[:, 0:384]
    FP[:, 384] = C[:, 384]                      # R_384 = sum x_d cos(pi d)
    FP[:, 385:768] = -S[:, 1:384]               # I_f = -sum x_d sin(...)
    w = np.full(NF, 2.0)
    w[0] = 1.0
    w[384] = 1.0
    GP = np.zeros((D, D), np.float32)
    GP[0:384, :] = (w[0:384, None] * C[:, 0:384].T) / D
    GP[384, :] = C[:, 384].T / D                # R_384 row
    GP[385:768, :] = -(2.0 * S[:, 1:384].T) / D  # I_f rows
    return FP, GP


def build_kernel(mm_dtype=F32R, reps=1, trace_sim=False):
    nc = bacc.Bacc("TRN2", target_bir_lowering=False, debug=False,
                   num_devices=NCORES)
    x = nc.dram_tensor("x", [BS, N, D], F32, kind="ExternalInput").ap()
    circ = nc.dram_tensor("circ", [2, D], F32, kind="ExternalInput").ap()
    positions = nc.dram_tensor("positions", [N, 2], I32,
                               kind="ExternalInput").ap()
    fp_c = nc.dram_tensor("fp_c", [D, D], F32, kind="ExternalInput").ap()
    gp_c = nc.dram_tensor("gp_c", [D, D], F32, kind="ExternalInput").ap()
    out = nc.dram_tensor("out", [BS, N, D], F32, kind="ExternalOutput").ap()

    with tile.TileContext(nc, trace_sim=trace_sim) as tc, ExitStack() as ctx:
        consts = ctx.enter_context(tc.tile_pool(name="consts", bufs=1))
        stage = ctx.enter_context(tc.tile_pool(name="stage", bufs=1))
        tabs = ctx.enter_context(tc.tile_pool(name="tabs", bufs=1))
        xio = ctx.enter_context(tc.tile_pool(name="xio", bufs=2))
        work = ctx.enter_context(tc.tile_pool(name="work", bufs=2))

        ident = consts.tile([P, P], F32, tag="ident")
        make_identity(nc, ident)

        # ---- load + round DFT matrices to fp32r ----
        FPt, GPt = [], []
        for name, src in (("fp", fp_c), ("gp", gp_c)):
            for c in range(NCH):
                st = stage.tile([P, D], F32, tag="mstage")
                nc.sync.dma_start(out=st, in_=src[c * P:(c + 1) * P, :])
                t = consts.tile([P, D], mm_dtype, tag=f"{name}{c}")
                nc.scalar.copy(out=t, in_=st)
                (FPt if name == "fp" else GPt).append(t)

        # ---- phase tables ----
        # posT (2, N) int32 -> *2.0 f32
        posT = tabs.tile([2, N], I32, tag="posT")
        nc.sync.dma_start(out=posT, in_=positions.rearrange("n k -> k n"))
        posTf = tabs.tile([2, N], F32, tag="posTf")
        nc.vector.tensor_scalar_mul(posTf, posT, 2.0)

        # circT (d on partitions): 6 chunks of (128, 2)
        circT = tabs.tile([P, 2 * NCH], F32, tag="circT")
        cr = circ.rearrange("k d -> d k")
        for c in range(NCH):
            nc.sync.dma_start(out=circT[:, 2 * c:2 * c + 2],
                              in_=cr[c * P:(c + 1) * P, :])

        # s2[k, f] = Im(rfft(circ_k))[f] for f=1..383 via FP imag columns
        ps0 = tc.tile_pool(name="ps0", bufs=1, space="PSUM")
        psum = ps0.__enter__()
        s2ps = psum.tile([2, 384], F32, tag="s2ps")
        for c in range(NCH):
            nc.tensor.matmul(s2ps[:, 1:384], circT[:, 2 * c:2 * c + 2],
                             FPt[c][:, 385:768].bitcast(F32),
                             start=(c == 0), stop=(c == NCH - 1))
        s2 = tabs.tile([2, 384], F32, tag="s2")
        nc.vector.memset(s2[:, 0:1], 0.0)
        nc.vector.tensor_copy(out=s2[:, 1:384], in_=s2ps[:, 1:384])

        # theta chunks (128 f, N) = s2_chunk^T @ posTf ; then sin/cos tables
        cT, sT = [], []
        for j in range(3):
            thps = psum.tile([P, N], F32, tag="thps")
            for h in range(2):
                nc.tensor.matmul(thps[:, h * 512:(h + 1) * 512],
                                 s2[:, j * P:(j + 1) * P],
                                 posTf[:, h * 512:(h + 1) * 512],
                                 start=True, stop=True)
            sj = tabs.tile([P, N], F32, tag=f"sT{j}")
            cj = tabs.tile([P, N], F32, tag=f"cT{j}")
            # sin: r = round(th/2pi); sin(th - 2pi r)
            t1 = stage.tile([P, N], F32, tag="pt")
            r1 = stage.tile([P, N], I32, tag="pr")
            u1 = stage.tile([P, N], F32, tag="pu")
            red = stage.tile([P, N], F32, tag="pred")
            nc.vector.tensor_scalar_mul(t1, thps, 1.0 / TWOPI)
            nc.vector.tensor_copy(out=r1, in_=t1)
            nc.vector.tensor_scalar_mul(u1, r1, -TWOPI)
            nc.vector.tensor_add(red, thps, u1)
            nc.scalar.activation(out=sj, in_=red,
                                 func=mybir.ActivationFunctionType.Sin)
            # cos: r2 = round(th/2pi + 1/4); cos = sin(th - 2pi r2 + pi/2)
            t2 = stage.tile([P, N], F32, tag="pt")
            r2 = stage.tile([P, N], I32, tag="pr")
            u2 = stage.tile([P, N], F32, tag="pu")
            red2 = stage.tile([P, N], F32, tag="pred")
            nc.vector.tensor_scalar(t2, thps, 1.0 / TWOPI, 0.25,
                                    op0=mybir.AluOpType.mult,
                                    op1=mybir.AluOpType.add)
            nc.vector.tensor_copy(out=r2, in_=t2)
            nc.vector.tensor_scalar(u2, r2, -TWOPI, math.pi / 2,
                                    op0=mybir.AluOpType.mult,
                                    op1=mybir.AluOpType.add)
            nc.vector.tensor_add(red2, thps, u2)
            nc.scalar.activation(out=cj, in_=red2,
                                 func=mybir.ActivationFunctionType.Sin)
            sT.append(sj)
            cT.append(cj)
        ps0.__exit__(None, None, None)

        # ---- main loop ----
        _ = reps
        pst_pool = ctx.enter_context(tc.tile_pool(name="pst", bufs=2, space="PSUM"))
        psf = ctx.enter_context(tc.tile_pool(name="psf", bufs=3, space="PSUM"))
        psi = ctx.enter_context(tc.tile_pool(name="psi", bufs=1, space="PSUM"))
        for rep in range(reps):
          for b in range(BS):
            for h in range(2):
                n0 = h * ROWTILE
                # load 4 row groups
                xg = []
                for g in range(NG):
                    t = xio.tile([P, D], F32, tag=f"x{g}")
                    nc.sync.dma_start(
                        out=t, in_=x[b, n0 + g * P:n0 + (g + 1) * P, :])
                    xg.append(t)
                # transpose to (d, rows) as 6 chunks of (128, 512), fp32r
                XT = []
                for c in range(NCH):
                    pst = pst_pool.tile([P, ROWTILE], F32, tag="pst")
                    for g in range(NG):
                        nc.tensor.transpose(pst[:, g * P:(g + 1) * P],
                                            xg[g][:, c * P:(c + 1) * P],
                                            ident)
                    xt = work.tile([P, ROWTILE], mm_dtype, tag=f"xt{c}")
                    nc.scalar.copy(out=xt, in_=pst)
                    XT.append(xt)
                # forward + rotation, pair (j, 3+j)
                RI = [None] * NCH
                for j in range(3):
                    pR = psf.tile([P, ROWTILE], F32, tag="psf")
                    pI = psf.tile([P, ROWTILE], F32, tag="psf")
                    for c in range(NCH):
                        nc.tensor.matmul(pR, FPt[c][:, j * P:(j + 1) * P],
                                         XT[c], start=(c == 0),
                                         stop=(c == NCH - 1))
                    for c in range(NCH):
                        nc.tensor.matmul(pI,
                                         FPt[c][:, (3 + j) * P:(4 + j) * P],
                                         XT[c], start=(c == 0),
                                         stop=(c == NCH - 1))
                    cs = cT[j][:, n0:n0 + ROWTILE]
                    sn = sT[j][:, n0:n0 + ROWTILE]
                    t1 = work.tile([P, ROWTILE], F32, tag="rta")
                    t2 = work.tile([P, ROWTILE], F32, tag="rtb")
                    nc.vector.tensor_mul(t1, pR, cs)
                    nc.vector.tensor_mul(t2, pI, sn)
                    rp = work.tile([P, ROWTILE], mm_dtype, tag=f"ri{j}")
                    ip = work.tile([P, ROWTILE], mm_dtype, tag=f"ri{3 + j}")
                    nc.gpsimd.tensor_sub(rp, t1, t2)
                    t3 = work.tile([P, ROWTILE], F32, tag="rta")
                    t4 = work.tile([P, ROWTILE], F32, tag="rtb")
                    nc.vector.tensor_mul(t3, pR, sn)
                    nc.vector.tensor_mul(t4, pI, cs)
                    nc.gpsimd.tensor_add(ip, t3, t4)
                    RI[j] = rp
                    RI[3 + j] = ip
                # inverse, per row group
                for g in range(NG):
                    pa = psi.tile([P, 512], F32, tag="pa")
                    pb = psi.tile([P, 256], F32, tag="pb")
                    for c in range(NCH):
                        lhs = RI[c][:, g * P:(g + 1) * P]
                        nc.tensor.matmul(pa, lhs, GPt[c][:, 0:512],
                                         start=(c == 0), stop=(c == NCH - 1))
                        nc.tensor.matmul(pb, lhs, GPt[c][:, 512:768],
                                         start=(c == 0), stop=(c == NCH - 1))
                    osb = xio.tile([P, D], F32, tag=f"o{g % 2}")
                    if g % 2 == 0:
                        nc.scalar.copy(out=osb[:, 0:512], in_=pa)
                        nc.scalar.copy(out=osb[:, 512:768], in_=pb)
                    else:
                        nc.vector.tensor_copy(out=osb[:, 0:512], in_=pa)
                        nc.vector.tensor_copy(out=osb[:, 512:768], in_=pb)
                    nc.sync.dma_start(
                        out=out[b, n0 + g * P:n0 + (g + 1) * P, :], in_=osb)
    nc.finalize()
    return nc


_NC_CACHE = {}


def kernel(x, circ, positions):
    x = np.ascontiguousarray(x, dtype=np.float32)
    circ = np.ascontiguousarray(circ, dtype=np.float32)
    positions = np.ascontiguousarray(positions, dtype=np.int32)
    if "nc" not in _NC_CACHE:
        _NC_CACHE["nc"] = build_kernel()
    nc = _NC_CACHE["nc"]
    FP, GP = _dft_matrices()
    in_maps = []
    for core in range(NCORES):
        in_maps.append({
            "x": x[core * BS:(core + 1) * BS],
            "circ": circ,
            "positions": positions,
            "fp_c": FP,
            "gp_c": GP,
        })
    res = bass_utils.run_bass_kernel_spmd(nc, in_maps,
                                          core_ids=list(range(NCORES)))
    out = np.concatenate([res.results[c]["out"] for c in range(NCORES)],
                         axis=0)
    return out


if __name__ == "__main__":
    rng = np.random.default_rng(0)
    x = rng.standard_normal((B, N, D)).astype(np.float32)
    circ = (rng.standard_normal((2, D)) * 0.01).astype(np.float32)
    positions = rng.integers(0, 32, (N, 2)).astype(np.int32)
    out = kernel(x=x, circ=circ, positions=positions)
    print("out", out.shape, out.dtype)


# revision 5
# speedup vs baseline: 19.4297x; 1.1233x over previous
"""Trainium2 Bass kernel for nn_CirculantSTRING.

Math: out[b,n,:] = irfft(exp(i*theta(n,:)) * rfft(x[b,n,:]), n=D)
where theta(n,f) = 2*(p0[n]*Im(rfft(circ0))[f] + p1[n]*Im(rfft(circ1))[f]).

Implementation per core (data-parallel over batch, 4 batches/core):
  - Forward rfft as a real matmul with packed output
    fi = [R_0..R_383 | R_384, I_1..I_383] (768 wide).
  - Phase rotation tables cos/sin(theta) computed on device from circ +
    positions (small matmuls + range-reduced Sin activation).
  - Elementwise rotation in (f partitions, rows free) layout.
  - Inverse rfft as a second real matmul back to (rows, d) layout.
Matmuls run in float32r (1 cycle/row on the PE at N>=256; ~11 mantissa
bits), phase tables and rotation in fp32.
"""
import math
from contextlib import ExitStack

import numpy as np

import concourse.bacc as bacc
import concourse.tile as tile
from concourse import mybir
from concourse import bass_utils
from concourse.masks import make_identity

F32 = mybir.dt.float32
F32R = mybir.dt.float32r
I32 = mybir.dt.int32

B, N, D = 32, 1024, 768
NCORES = 8
BS = B // NCORES          # batches per core
P = 128                   # partitions
NF = D // 2 + 1           # 385 rfft freqs
NCH = D // P              # 6 chunks of 128 along d / fi
ROWTILE = 512             # rows processed per main-loop tile (one b, half of N)
NG = ROWTILE // P         # 4 row groups per tile

TWOPI = 2.0 * math.pi


def _dft_matrices():
    """Forward FP (d, fi) and inverse GP (fi, d) packed real-DFT matrices.

    fi packing: [R_0..R_383 | R_384, I_1..I_383].
    """
    d = np.arange(D)
    f = np.arange(NF)
    ang = 2.0 * np.pi * np.outer(d, f) / D      # (768, 385)
    C = np.cos(ang)
    S = np.sin(ang)
    FP = np.zeros((D, D), np.float32)
    FP[:, 0:384] = C[:, 0:384]
    FP[:, 384] = C[:, 384]                      # R_384 = sum x_d cos(pi d)
    FP[:, 385:768] = -S[:, 1:384]               # I_f = -sum x_d sin(...)
    w = np.full(NF, 2.0)
    w[0] = 1.0
    w[384] = 1.0
    GP = np.zeros((D, D), np.float32)
    GP[0:384, :] = (w[0:384, None] * C[:, 0:384].T) / D
    GP[384, :] = C[:, 384].T / D                # R_384 row
    GP[385:768, :] = -(2.0 * S[:, 1:384].T) / D  # I_f rows
    return FP, GP


def build_kernel(mm_dtype=F32R, reps=1, trace_sim=False):
    nc = bacc.Bacc("TRN2", target_bir_lowering=False, debug=False,
                   num_devices=NCORES)
    x = nc.dram_tensor("x", [BS, N, D], F32, kind="ExternalInput").ap()
    circ = nc.dram_tensor("circ", [2, D], F32, kind="ExternalInput").ap()
    positions = nc.dram_tensor("positions", [N, 2], I32,
                               kind="ExternalInput").ap()
    fp_c = nc.dram_tensor("fp_c", [D, D], F32, kind="ExternalInput").ap()
    gp_c = nc.dram_tensor("gp_c", [D, D], F32, kind="ExternalInput").ap()
    out = nc.dram_tensor("out", [BS, N, D], F32, kind="ExternalOutput").ap()

    with tile.TileContext(nc, trace_sim=trace_sim) as tc, ExitStack() as ctx:
        consts = ctx.enter_context(tc.tile_pool(name="consts", bufs=1))
        stage = ctx.enter_context(tc.tile_pool(name="stage", bufs=1))
        tabs = ctx.enter_context(tc.tile_pool(name="tabs", bufs=1))
        xio = ctx.enter_context(tc.tile_pool(name="xio", bufs=2))
        work = ctx.enter_context(tc.tile_pool(name="work", bufs=2))

        ident = consts.tile([P, P], F32, tag="ident")
        make_identity(nc, ident)

        # ---- load + round DFT matrices to fp32r ----
        FPt, GPt = [], []
        for name, src in (("fp", fp_c), ("gp", gp_c)):
            for c in range(NCH):
                st = stage.tile([P, D], F32, tag="mstage")
                nc.sync.dma_start(out=st, in_=src[c * P:(c + 1) * P, :])
                t = consts.tile([P, D], mm_dtype, tag=f"{name}{c}")
                nc.scalar.copy(out=t, in_=st)
                (FPt if name == "fp" else GPt).append(t)

        # ---- phase tables ----
        # posT (2, N) int32 -> *2.0 f32
        posT = tabs.tile([2, N], I32, tag="posT")
        nc.sync.dma_start(out=posT, in_=positions.rearrange("n k -> k n"))
        posTf = tabs.tile([2, N], F32, tag="posTf")
        nc.vector.tensor_scalar_mul(posTf, posT, 2.0)

        # circT (d on partitions): 6 chunks of (128, 2)
        circT = tabs.tile([P, 2 * NCH], F32, tag="circT")
        cr = circ.rearrange("k d -> d k")
        for c in range(NCH):
            nc.sync.dma_start(out=circT[:, 2 * c:2 * c + 2],
                              in_=cr[c * P:(c + 1) * P, :])

        # s2[k, f] = Im(rfft(circ_k))[f] for f=1..383 via FP imag columns
        ps0 = tc.tile_pool(name="ps0", bufs=1, space="PSUM")
        psum = ps0.__enter__()
        s2ps = psum.tile([2, 384], F32, tag="s2ps")
        for c in range(NCH):
            nc.tensor.matmul(s2ps[:, 1:384], circT[:, 2 * c:2 * c + 2],
                             FPt[c][:, 385:768].bitcast(F32),
                             start=(c == 0), stop=(c == NCH - 1))
        s2 = tabs.tile([2, 384], F32, tag="s2")
        nc.vector.memset(s2[:, 0:1], 0.0)
        nc.vector.tensor_copy(out=s2[:, 1:384], in_=s2ps[:, 1:384])

        # theta chunks (128 f, N) = s2_chunk^T @ posTf ; then sin/cos tables
        cT, sT = [], []
        for j in range(3):
            thps = psum.tile([P, N], F32, tag="thps")
            for h in range(2):
                nc.tensor.matmul(thps[:, h * 512:(h + 1) * 512],
                                 s2[:, j * P:(j + 1) * P],
                                 posTf[:, h * 512:(h + 1) * 512],
                                 start=True, stop=True)
            sj = tabs.tile([P, N], F32, tag=f"sT{j}")
            cj = tabs.tile([P, N], F32, tag=f"cT{j}")
            # sin: r = round(th/2pi); sin(th - 2pi r)
            t1 = stage.tile([P, N], F32, tag="pt")
            r1 = stage.tile([P, N], I32, tag="pr")
            u1 = stage.tile([P, N], F32, tag="pu")
            red = stage.tile([P, N], F32, tag="pred")
            nc.vector.tensor_scalar_mul(t1, thps, 1.0 / TWOPI)
            nc.vector.tensor_copy(out=r1, in_=t1)
            nc.vector.tensor_scalar_mul(u1, r1, -TWOPI)
            nc.vector.tensor_add(red, thps, u1)
            nc.scalar.activation(out=sj, in_=red,
                                 func=mybir.ActivationFunctionType.Sin)
            # cos: r2 = round(th/2pi + 1/4); cos = sin(th - 2pi r2 + pi/2)
            t2 = stage.tile([P, N], F32, tag="pt")
            r2 = stage.tile([P, N], I32, tag="pr")
            u2 = stage.tile([P, N], F32, tag="pu")
            red2 = stage.tile([P, N], F32, tag="pred")
            nc.vector.tensor_scalar(t2, thps, 1.0 / TWOPI, 0.25,
                                    op0=mybir.AluOpType.mult,
                                    op1=mybir.AluOpType.add)
            nc.vector.tensor_copy(out=r2, in_=t2)
            nc.vector.tensor_scalar(u2, r2, -TWOPI, math.pi / 2,
                                    op0=mybir.AluOpType.mult,
                                    op1=mybir.AluOpType.add)
            nc.vector.tensor_add(red2, thps, u2)
            nc.scalar.activation(out=cj, in_=red2,
                                 func=mybir.ActivationFunctionType.Sin)
            sT.append(sj)
            cT.append(cj)
        ps0.__exit__(None, None, None)

        # ---- main loop ----
        _ = reps
        pst_pool = ctx.enter_context(tc.tile_pool(name="pst", bufs=2, space="PSUM"))
        psf = ctx.enter_context(tc.tile_pool(name="psf", bufs=3, space="PSUM"))
        psi = ctx.enter_context(tc.tile_pool(name="psi", bufs=1, space="PSUM"))
        for rep in range(reps):
          for b in range(BS):
            for h in range(2):
                n0 = h * ROWTILE
                # load 4 row groups
                xg = []
                for g in range(NG):
                    t = xio.tile([P, D], F32, tag=f"x{g}")
                    nc.sync.dma_start(
                        out=t, in_=x[b, n0 + g * P:n0 + (g + 1) * P, :])
                    xg.append(t)
                # transpose to (d, rows) as 6 chunks of (128, 512), fp32r
                XT = []
                for c in range(NCH):
                    pst = pst_pool.tile([P, ROWTILE], F32, tag="pst")
                    for g in range(NG):
                        nc.tensor.transpose(pst[:, g * P:(g + 1) * P],
                                            xg[g][:, c * P:(c + 1) * P],
                                            ident)
                    xt = work.tile([P, ROWTILE], mm_dtype, tag=f"xt{c}")
                    nc.scalar.copy(out=xt, in_=pst)
                    XT.append(xt)
                # forward + rotation, pair (j, 3+j)
                RI = [None] * NCH
                for j in range(3):
                    pR = psf.tile([P, ROWTILE], F32, tag="psf")
                    pI = psf.tile([P, ROWTILE], F32, tag="psf")
                    for c in range(NCH):
                        nc.tensor.matmul(pR, FPt[c][:, j * P:(j + 1) * P],
                                         XT[c], start=(c == 0),
                                         stop=(c == NCH - 1))
                    for c in range(NCH):
                        nc.tensor.matmul(pI,
                                         FPt[c][:, (3 + j) * P:(4 + j) * P],
                                         XT[c], start=(c == 0),
                                         stop=(c == NCH - 1))
                    cs = cT[j][:, n0:n0 + ROWTILE]
                    sn = sT[j][:, n0:n0 + ROWTILE]
                    # evacuate psum -> sbuf on ACT so psf recycles fast and
                    # Pool (no PSUM access) can share rotation work
                    sR = work.tile([P, ROWTILE], F32, tag="sR")
                    sI = work.tile([P, ROWTILE], F32, tag="sI")
                    nc.scalar.copy(out=sR, in_=pR)
                    nc.scalar.copy(out=sI, in_=pI)
                    t1 = work.tile([P, ROWTILE], F32, tag="rta")
                    t2 = work.tile([P, ROWTILE], F32, tag="rtb")
                    t3 = work.tile([P, ROWTILE], F32, tag="rtc")
                    t4 = work.tile([P, ROWTILE], F32, tag="rtd")
                    nc.vector.tensor_mul(t1, sR, cs)
                    nc.gpsimd.tensor_mul(t2, sI, sn)
                    nc.vector.tensor_mul(t4, sI, cs)
                    nc.gpsimd.tensor_mul(t3, sR, sn)
                    rp = work.tile([P, ROWTILE], mm_dtype, tag=f"ri{j}")
                    ip = work.tile([P, ROWTILE], mm_dtype, tag=f"ri{3 + j}")
                    nc.vector.tensor_sub(rp, t1, t2)
                    nc.gpsimd.tensor_add(ip, t3, t4)
                    RI[j] = rp
                    RI[3 + j] = ip
                # inverse, per row group
                for g in range(NG):
                    pa = psi.tile([P, 512], F32, tag="pa")
                    pb = psi.tile([P, 256], F32, tag="pb")
                    for c in range(NCH):
                        lhs = RI[c][:, g * P:(g + 1) * P]
                        nc.tensor.matmul(pa, lhs, GPt[c][:, 0:512],
                                         start=(c == 0), stop=(c == NCH - 1))
                        nc.tensor.matmul(pb, lhs, GPt[c][:, 512:768],
                                         start=(c == 0), stop=(c == NCH - 1))
                    osb = xio.tile([P, D], F32, tag=f"o{g % 2}")
                    if g % 2 == 0:
                        nc.scalar.copy(out=osb[:, 0:512], in_=pa)
                        nc.scalar.copy(out=osb[:, 512:768], in_=pb)
                    else:
                        nc.vector.tensor_copy(out=osb[:, 0:512], in_=pa)
                        nc.vector.tensor_copy(out=osb[:, 512:768], in_=pb)
                    nc.sync.dma_start(
                        out=out[b, n0 + g * P:n0 + (g + 1) * P, :], in_=osb)
    nc.finalize()
    return nc


_NC_CACHE = {}


def kernel(x, circ, positions):
    x = np.ascontiguousarray(x, dtype=np.float32)
    circ = np.ascontiguousarray(circ, dtype=np.float32)
    positions = np.ascontiguousarray(positions, dtype=np.int32)
    if "nc" not in _NC_CACHE:
        _NC_CACHE["nc"] = build_kernel()
    nc = _NC_CACHE["nc"]
    FP, GP = _dft_matrices()
    in_maps = []
    for core in range(NCORES):
        in_maps.append({
            "x": x[core * BS:(core + 1) * BS],
            "circ": circ,
            "positions": positions,
            "fp_c": FP,
            "gp_c": GP,
        })
    res = bass_utils.run_bass_kernel_spmd(nc, in_maps,
                                          core_ids=list(range(NCORES)))
    out = np.concatenate([res.results[c]["out"] for c in range(NCORES)],
                         axis=0)
    return out


if __name__ == "__main__":
    rng = np.random.default_rng(0)
    x = rng.standard_normal((B, N, D)).astype(np.float32)
    circ = (rng.standard_normal((2, D)) * 0.01).astype(np.float32)
    positions = rng.integers(0, 32, (N, 2)).astype(np.int32)
    out = kernel(x=x, circ=circ, positions=positions)
    print("out", out.shape, out.dtype)


# revision 11
# speedup vs baseline: 36.2790x; 1.8672x over previous
"""Trainium2 Bass kernel for nn_CirculantSTRING.

Math: out[b,n,:] = irfft(exp(i*theta(n,:)) * rfft(x[b,n,:]), n=D)
where theta(n,f) = 2*(p0[n]*Im(rfft(circ0))[f] + p1[n]*Im(rfft(circ1))[f]).

Per core (data-parallel over batch, 4 batches/core), folded real-DFT:
  - even/odd fold: eo = [e_0..e_383 | x_384, o_1..o_383],
    e_d = x_d + x_{768-d}, o_d = x_d - x_{768-d} (halves forward matmul work)
  - forward matmul to fi = [R_0..R_383 | R_384, I_1..I_383]
    (block-sparse F2: 24 of 36 (128,128) blocks)
  - phase rotation with on-device cos/sin(theta) tables; theta(n,0)=0 makes
    the R_384 slot (chunk 3, partition 0) pass through untouched
  - inverse matmul to uv = [u_0..u_384 | v_1..v_383] (7 matmuls/row-group),
    un-fold out[d] = u_d - v_d, out[768-d] = u_d + v_d
Matmuls in float32r (1 cyc/row on PE at N>=256, ~11 mantissa bits); phase
path in fp32.
"""
import math
from contextlib import ExitStack

import numpy as np

import concourse.bacc as bacc
import concourse.tile as tile
from concourse import mybir
from concourse import bass_utils
from concourse.masks import make_identity

F32 = mybir.dt.float32
F32R = mybir.dt.float32r
I32 = mybir.dt.int32

B, N, D = 32, 1024, 768
NCORES = 8
BS = B // NCORES
P = 128
NCH = D // P              # 6
ROWTILE = 512
NG = ROWTILE // P         # 4

TWOPI = 2.0 * math.pi

# forward block list: M-chunk -> list of K-chunks
FWD_BLOCKS = {0: [0, 1, 2, 3], 1: [0, 1, 2, 3], 2: [0, 1, 2, 3],
              3: [0, 1, 2, 3, 4, 5], 4: [3, 4, 5], 5: [3, 4, 5]}


def _dft_matrices():
    """Folded forward F2 (eo -> fi) and inverse G2 (fi -> uv)."""
    dd = np.arange(384)
    ff = np.arange(384)
    fo = np.arange(1, 384)
    du = np.arange(385)
    F2 = np.zeros((D, D), np.float32)
    F2[0:384, 0:384] = np.cos(2 * np.pi * np.outer(dd, ff) / D)
    F2[0:384, 384] = (-1.0) ** dd
    F2[384, 0:384] = (-1.0) ** ff
    F2[384, 384] = 1.0
    F2[385:768, 385:768] = -np.sin(2 * np.pi * np.outer(fo, fo) / D)
    w = np.full(385, 2.0)
    w[0] = 1.0
    w[384] = 1.0
    G2 = np.zeros((D, 770), np.float32)
    G2[0:384, 0:385] = (w[0:384, None]
                        * np.cos(2 * np.pi * np.outer(ff, du) / D)) / D
    G2[384, 0:385] = ((-1.0) ** du) / D
    G2[385:768, 386:769] = (2.0 * np.sin(2 * np.pi * np.outer(fo, fo) / D)) / D
    return F2, G2


def build_kernel(mm_dtype=F32R, reps=1, trace_sim=False):
    nc = bacc.Bacc("TRN2", target_bir_lowering=False, debug=False,
                   num_devices=NCORES)
    x = nc.dram_tensor("x", [BS, N, D], F32, kind="ExternalInput").ap()
    circ = nc.dram_tensor("circ", [2, D], F32, kind="ExternalInput").ap()
    positions = nc.dram_tensor("positions", [N, 2], I32,
                               kind="ExternalInput").ap()
    fp_c = nc.dram_tensor("fp_c", [D, D], F32, kind="ExternalInput").ap()
    gp_c = nc.dram_tensor("gp_c", [D, 770], F32, kind="ExternalInput").ap()
    out = nc.dram_tensor("out", [BS, N, D], F32, kind="ExternalOutput").ap()

    with tile.TileContext(nc, trace_sim=trace_sim) as tc, ExitStack() as ctx:
        consts = ctx.enter_context(tc.tile_pool(name="consts", bufs=1))
        stage = ctx.enter_context(tc.tile_pool(name="stage", bufs=1))
        tabs = ctx.enter_context(tc.tile_pool(name="tabs", bufs=1))
        xio = ctx.enter_context(tc.tile_pool(name="xio", bufs=2))
        work = ctx.enter_context(tc.tile_pool(name="work", bufs=2))

        ident = consts.tile([P, P], F32, tag="ident")
        make_identity(nc, ident)

        ps0 = tc.tile_pool(name="ps0", bufs=1, space="PSUM")
        psum = ps0.__enter__()

        # ---- circ odd-fold (for s2, in fp32) ----
        circR = tabs.tile([2, D], F32, tag="circR")
        nc.sync.dma_start(out=circR, in_=circ)
        ocr = tabs.tile([2, 384], F32, tag="ocr")
        nc.vector.memset(ocr[:, 0:1], 0.0)
        nc.vector.tensor_sub(ocr[:, 1:384], circR[:, 1:384],
                             circR[:, 767:384:-1])
        occ = []  # (128, 2) fp32, o-fold of circ on chunk 3..5 partitions
        for i in range(3):
            poc = psum.tile([P, 2], F32, tag="pocc")
            nc.tensor.transpose(poc, ocr[:, i * P:(i + 1) * P], ident[0:2, 0:2])
            so = tabs.tile([P, 2], F32, tag=f"occ{i}")
            nc.scalar.copy(out=so, in_=poc)
            occ.append(so)

        # ---- load DFT matrices; s2 matmul on fp32 staging of F2 chunks 3..5
        s2ps = psum.tile([2, 384], F32, tag="s2ps")
        FPt, GPt = [], []
        for name, src in (("fp", fp_c), ("gp", gp_c)):
            for c in range(NCH):
                st = stage.tile([P, 770], F32, tag="mstage")
                nc.sync.dma_start(out=st[:, 0:src.shape[1]], in_=src[c * P:(c + 1) * P, :])
                if name == "fp" and c >= 3:
                    nc.tensor.matmul(s2ps[:, 1:384], occ[c - 3],
                                     st[:, 385:768],
                                     start=(c == 3), stop=(c == 5))
                wdt = D if name == "fp" else 770
                t = consts.tile([P, wdt], mm_dtype, tag=f"{name}{c}")
                nc.scalar.copy(out=t, in_=st[:, 0:wdt])
                (FPt if name == "fp" else GPt).append(t)
        s2 = tabs.tile([2, 384], F32, tag="s2")
        nc.vector.memset(s2[:, 0:1], 0.0)
        nc.vector.tensor_copy(out=s2[:, 1:384], in_=s2ps[:, 1:384])

        # ---- positions ----
        posT = tabs.tile([2, N], I32, tag="posT")
        nc.sync.dma_start(out=posT, in_=positions.rearrange("n k -> k n"))
        posTf = tabs.tile([2, N], F32, tag="posTf")
        nc.vector.tensor_scalar_mul(posTf, posT, 2.0)

        # ---- theta -> cos/sin tables (3 chunks of (128, N)) ----
        cT, sT = [], []
        for j in range(3):
            thps = psum.tile([P, N], F32, tag="thps")
            for h in range(2):
                nc.tensor.matmul(thps[:, h * 512:(h + 1) * 512],
                                 s2[:, j * P:(j + 1) * P],
                                 posTf[:, h * 512:(h + 1) * 512],
                                 start=True, stop=True)
            sj = tabs.tile([P, N], F32, tag=f"sT{j}")
            cj = tabs.tile([P, N], F32, tag=f"cT{j}")
            t1 = stage.tile([P, N], F32, tag="pt")
            r1 = stage.tile([P, N], I32, tag="pr")
            u1 = stage.tile([P, N], F32, tag="pu")
            red = stage.tile([P, N], F32, tag="pred")
            nc.vector.tensor_scalar_mul(t1, thps, 1.0 / TWOPI)
            nc.vector.tensor_copy(out=r1, in_=t1)
            nc.vector.tensor_scalar_mul(u1, r1, -TWOPI)
            nc.vector.tensor_add(red, thps, u1)
            nc.scalar.activation(out=sj, in_=red,
                                 func=mybir.ActivationFunctionType.Sin)
            t2 = stage.tile([P, N], F32, tag="pt")
            r2 = stage.tile([P, N], I32, tag="pr")
            u2 = stage.tile([P, N], F32, tag="pu")
            red2 = stage.tile([P, N], F32, tag="pred")
            nc.vector.tensor_scalar(t2, thps, 1.0 / TWOPI, 0.25,
                                    op0=mybir.AluOpType.mult,
                                    op1=mybir.AluOpType.add)
            nc.vector.tensor_copy(out=r2, in_=t2)
            nc.vector.tensor_scalar(u2, r2, -TWOPI, math.pi / 2,
                                    op0=mybir.AluOpType.mult,
                                    op1=mybir.AluOpType.add)
            nc.vector.tensor_add(red2, thps, u2)
            nc.scalar.activation(out=cj, in_=red2,
                                 func=mybir.ActivationFunctionType.Sin)
            sT.append(sj)
            cT.append(cj)
        ps0.__exit__(None, None, None)

        # ---- main loop ----
        pst_pool = ctx.enter_context(tc.tile_pool(name="pst", bufs=1,
                                                  space="PSUM"))
        psf = ctx.enter_context(tc.tile_pool(name="psf", bufs=3, space="PSUM"))
        psi = ctx.enter_context(tc.tile_pool(name="psi", bufs=2, space="PSUM"))
        for rep in range(reps):
          for b in range(BS):
            for h in range(2):
                n0 = h * ROWTILE
                # load 4 row groups; even/odd fold on Pool/DVE
                eog = []
                for g in range(NG):
                    t = xio.tile([P, D], F32, tag=f"x{g}")
                    nc.sync.dma_start(
                        out=t, in_=x[b, n0 + g * P:n0 + (g + 1) * P, :])
                    eo = xio.tile([P, D], F32, tag=f"eo{g}")
                    nc.gpsimd.tensor_add(eo[:, 1:384], t[:, 1:384],
                                         t[:, 767:384:-1])
                    nc.gpsimd.tensor_sub(eo[:, 385:768], t[:, 1:384],
                                         t[:, 767:384:-1])
                    nc.vector.tensor_copy(out=eo[:, 0:385:384],
                                          in_=t[:, 0:385:384])
                    eog.append(eo)
                # transpose eo to (d', rows): 6 chunks of (128, 512), fp32r
                XT = []
                for c in range(NCH):
                    pst = pst_pool.tile([P, ROWTILE], F32, tag="pst")
                    for g in range(NG):
                        nc.tensor.transpose(pst[:, g * P:(g + 1) * P],
                                            eog[g][:, c * P:(c + 1) * P],
                                            ident)
                    xt = work.tile([P, ROWTILE], mm_dtype, tag=f"xt{c}")
                    nc.scalar.copy(out=xt, in_=pst)
                    XT.append(xt)
                # forward (block-sparse) + rotation per pair (j, 3+j)
                RI = [None] * NCH
                for j in range(3):
                    pR = psf.tile([P, ROWTILE], F32, tag="psf")
                    pI = psf.tile([P, ROWTILE], F32, tag="psf")
                    kR = FWD_BLOCKS[j]
                    for i, c in enumerate(kR):
                        nc.tensor.matmul(pR, FPt[c][:, j * P:(j + 1) * P],
                                         XT[c], start=(i == 0),
                                         stop=(i == len(kR) - 1))
                    kI = FWD_BLOCKS[3 + j]
                    for i, c in enumerate(kI):
                        nc.tensor.matmul(pI,
                                         FPt[c][:, (3 + j) * P:(4 + j) * P],
                                         XT[c], start=(i == 0),
                                         stop=(i == len(kI) - 1))
                    cs = cT[j][:, n0:n0 + ROWTILE]
                    sn = sT[j][:, n0:n0 + ROWTILE]
                    t1 = work.tile([P, ROWTILE], F32, tag="rta")
                    t2 = work.tile([P, ROWTILE], F32, tag="rtb")
                    t3 = work.tile([P, ROWTILE], F32, tag="rtc")
                    t4 = work.tile([P, ROWTILE], F32, tag="rtd")
                    nc.vector.tensor_mul(t1, pR, cs)
                    nc.vector.tensor_mul(t3, pR, sn)
                    nc.vector.tensor_mul(t2, pI, sn)
                    nc.vector.tensor_mul(t4, pI, cs)
                    rp = work.tile([P, ROWTILE], mm_dtype, tag=f"ri{j}")
                    ip = work.tile([P, ROWTILE], mm_dtype, tag=f"ri{3 + j}")
                    nc.gpsimd.tensor_sub(rp, t1, t2)
                    nc.gpsimd.tensor_add(ip, t3, t4)
                    RI[j] = rp
                    RI[3 + j] = ip
                # inverse (folded): u (385) and v (383) psum, un-fold to osb
                for g in range(NG):
                    pa = psi.tile([P, 386], F32, tag="pa")
                    pb = psi.tile([P, 384], F32, tag="pb")
                    gs = slice(g * P, (g + 1) * P)
                    for i, c in enumerate((0, 1, 2, 3)):
                        nc.tensor.matmul(pa, RI[c][:, gs], GPt[c][:, 0:386],
                                         start=(i == 0), stop=(i == 3))
                    for i, c in enumerate((3, 4, 5)):
                        nc.tensor.matmul(pb, RI[c][:, gs], GPt[c][:, 386:770],
                                         start=(i == 0), stop=(i == 2))
                    vb = work.tile([P, 384], F32, tag="rta")
                    nc.scalar.copy(out=vb, in_=pb)
                    osb = xio.tile([P, D], F32, tag=f"eo{g}")
                    nc.vector.tensor_sub(osb[:, 1:384], pa[:, 1:384],
                                         vb[:, 0:383])
                    nc.vector.tensor_add(osb[:, 385:768], pa[:, 383:0:-1],
                                         vb[:, 382::-1])
                    nc.vector.tensor_copy(out=osb[:, 0:385:384],
                                          in_=pa[:, 0:385:384])
                    nc.sync.dma_start(
                        out=out[b, n0 + g * P:n0 + (g + 1) * P, :], in_=osb)
    nc.finalize()
    return nc


_NC_CACHE = {}


def kernel(x, circ, positions):
    x = np.ascontiguousarray(x, dtype=np.float32)
    circ = np.ascontiguousarray(circ, dtype=np.float32)
    positions = np.ascontiguousarray(positions, dtype=np.int32)
    if "nc" not in _NC_CACHE:
        _NC_CACHE["nc"] = build_kernel()
    nc = _NC_CACHE["nc"]
    FP, GP = _dft_matrices()
    in_maps = []
    for core in range(NCORES):
        in_maps.append({
            "x": x[core * BS:(core + 1) * BS],
            "circ": circ,
            "positions": positions,
            "fp_c": FP,
            "gp_c": GP,
        })
    res = bass_utils.run_bass_kernel_spmd(nc, in_maps,
                                          core_ids=list(range(NCORES)))
    out = np.concatenate([res.results[c]["out"] for c in range(NCORES)],
                         axis=0)
    return out


if __name__ == "__main__":
    rng = np.random.default_rng(0)
    x = rng.standard_normal((B, N, D)).astype(np.float32)
    circ = (rng.standard_normal((2, D)) * 0.01).astype(np.float32)
    positions = rng.integers(0, 32, (N, 2)).astype(np.int32)
    out = kernel(x=x, circ=circ, positions=positions)
    print("out", out.shape, out.dtype)


# revision 13
# speedup vs baseline: 61.8409x; 1.7046x over previous
"""Trainium2 Bass kernel for nn_CirculantSTRING.

Math: out[b,n,:] = irfft(exp(i*theta(n,:)) * rfft(x[b,n,:]), n=D)
where theta(n,f) = 2*(p0[n]*Im(rfft(circ0))[f] + p1[n]*Im(rfft(circ1))[f]).

Per core (data-parallel over batch, 4 batches/core), folded real-DFT:
  - even/odd fold: eo = [e_0..e_383 | x_384, o_1..o_383],
    e_d = x_d + x_{768-d}, o_d = x_d - x_{768-d} (halves forward matmul work)
  - forward matmul to fi = [R_0..R_383 | R_384, I_1..I_383]
    (block-sparse F2: 24 of 36 (128,128) blocks)
  - phase rotation with on-device cos/sin(theta) tables; theta(n,0)=0 makes
    the R_384 slot (chunk 3, partition 0) pass through untouched
  - inverse matmul to uv = [u_0..u_384 | v_1..v_383] (7 matmuls/row-group),
    un-fold out[d] = u_d - v_d, out[768-d] = u_d + v_d
Matmuls in float32r (1 cyc/row on PE at N>=256, ~11 mantissa bits); phase
path in fp32.
"""
import math
from contextlib import ExitStack

import numpy as np

import concourse.bacc as bacc
import concourse.tile as tile
from concourse import mybir
from concourse import bass_utils
from concourse.masks import make_identity

F32 = mybir.dt.float32
F32R = mybir.dt.float32r
I32 = mybir.dt.int32

B, N, D = 32, 1024, 768
NCORES = 8
BS = B // NCORES
P = 128
NCH = D // P              # 6
ROWTILE = 512
NG = ROWTILE // P         # 4

TWOPI = 2.0 * math.pi

# forward block list: M-chunk -> list of K-chunks
FWD_BLOCKS = {0: [0, 1, 2, 3], 1: [0, 1, 2, 3], 2: [0, 1, 2, 3],
              3: [0, 1, 2, 3, 4, 5], 4: [3, 4, 5], 5: [3, 4, 5]}


def _dft_matrices():
    """Folded forward F2 (eo -> fi) and inverse G2 (fi -> uv)."""
    dd = np.arange(384)
    ff = np.arange(384)
    fo = np.arange(1, 384)
    du = np.arange(385)
    F2 = np.zeros((D, D), np.float32)
    F2[0:384, 0:384] = np.cos(2 * np.pi * np.outer(dd, ff) / D)
    F2[0:384, 384] = (-1.0) ** dd
    F2[384, 0:384] = (-1.0) ** ff
    F2[384, 384] = 1.0
    F2[385:768, 385:768] = -np.sin(2 * np.pi * np.outer(fo, fo) / D)
    w = np.full(385, 2.0)
    w[0] = 1.0
    w[384] = 1.0
    G2 = np.zeros((D, 770), np.float32)
    G2[0:384, 0:385] = (w[0:384, None]
                        * np.cos(2 * np.pi * np.outer(ff, du) / D)) / D
    G2[384, 0:385] = ((-1.0) ** du) / D
    G2[385:768, 386:769] = (2.0 * np.sin(2 * np.pi * np.outer(fo, fo) / D)) / D
    return F2, G2


def build_kernel(mm_dtype=F32R, reps=1, trace_sim=False):
    nc = bacc.Bacc("TRN2", target_bir_lowering=False, debug=False,
                   num_devices=NCORES)
    x = nc.dram_tensor("x", [BS, N, D], F32, kind="ExternalInput").ap()
    circ = nc.dram_tensor("circ", [2, D], F32, kind="ExternalInput").ap()
    positions = nc.dram_tensor("positions", [N, 2], I32,
                               kind="ExternalInput").ap()
    fp_c = nc.dram_tensor("fp_c", [D, D], F32, kind="ExternalInput").ap()
    gp_c = nc.dram_tensor("gp_c", [D, 770], F32, kind="ExternalInput").ap()
    out = nc.dram_tensor("out", [BS, N, D], F32, kind="ExternalOutput").ap()

    with tile.TileContext(nc, trace_sim=trace_sim) as tc, ExitStack() as ctx:
        consts = ctx.enter_context(tc.tile_pool(name="consts", bufs=1))
        stage = ctx.enter_context(tc.tile_pool(name="stage", bufs=1))
        tabs = ctx.enter_context(tc.tile_pool(name="tabs", bufs=1))
        xio = ctx.enter_context(tc.tile_pool(name="xio", bufs=2))
        work = ctx.enter_context(tc.tile_pool(name="work", bufs=2))

        ident = consts.tile([P, P], F32, tag="ident")
        make_identity(nc, ident)

        ps0 = tc.tile_pool(name="ps0", bufs=1, space="PSUM")
        psum = ps0.__enter__()

        # ---- circ odd-fold (for s2, in fp32) ----
        circR = tabs.tile([2, D], F32, tag="circR")
        nc.sync.dma_start(out=circR, in_=circ)
        ocr = tabs.tile([2, 384], F32, tag="ocr")
        nc.vector.memset(ocr[:, 0:1], 0.0)
        nc.vector.tensor_sub(ocr[:, 1:384], circR[:, 1:384],
                             circR[:, 767:384:-1])
        occ = []  # (128, 2) fp32, o-fold of circ on chunk 3..5 partitions
        for i in range(3):
            poc = psum.tile([P, 2], F32, tag="pocc")
            nc.tensor.transpose(poc, ocr[:, i * P:(i + 1) * P], ident[0:2, 0:2])
            so = tabs.tile([P, 2], F32, tag=f"occ{i}")
            nc.scalar.copy(out=so, in_=poc)
            occ.append(so)

        # ---- load DFT matrices; s2 matmul on fp32 staging of F2 chunks 3..5
        s2ps = psum.tile([2, 384], F32, tag="s2ps")
        FPt, GPt = [], []
        for name, src in (("fp", fp_c), ("gp", gp_c)):
            for c in range(NCH):
                st = stage.tile([P, 770], F32, tag="mstage")
                nc.sync.dma_start(out=st[:, 0:src.shape[1]], in_=src[c * P:(c + 1) * P, :])
                if name == "fp" and c >= 3:
                    nc.tensor.matmul(s2ps[:, 1:384], occ[c - 3],
                                     st[:, 385:768],
                                     start=(c == 3), stop=(c == 5))
                wdt = D if name == "fp" else 770
                t = consts.tile([P, wdt], mm_dtype, tag=f"{name}{c}")
                nc.scalar.copy(out=t, in_=st[:, 0:wdt])
                (FPt if name == "fp" else GPt).append(t)
        s2 = tabs.tile([2, 384], F32, tag="s2")
        nc.vector.memset(s2[:, 0:1], 0.0)
        nc.vector.tensor_copy(out=s2[:, 1:384], in_=s2ps[:, 1:384])

        # ---- positions ----
        posT = tabs.tile([2, N], I32, tag="posT")
        nc.sync.dma_start(out=posT, in_=positions.rearrange("n k -> k n"))
        posTf = tabs.tile([2, N], F32, tag="posTf")
        nc.vector.tensor_scalar_mul(posTf, posT, 2.0)

        # ---- theta -> cos/sin tables (3 chunks of (128, N)) ----
        cT, sT = [], []
        for j in range(3):
            thps = psum.tile([P, N], F32, tag="thps")
            for h in range(2):
                nc.tensor.matmul(thps[:, h * 512:(h + 1) * 512],
                                 s2[:, j * P:(j + 1) * P],
                                 posTf[:, h * 512:(h + 1) * 512],
                                 start=True, stop=True)
            sj = tabs.tile([P, N], F32, tag=f"sT{j}")
            cj = tabs.tile([P, N], F32, tag=f"cT{j}")
            for hh in range(2):
                hs = slice(hh * 512, (hh + 1) * 512)
                te = stage.tile([P, 512], F32, tag="te")
                nc.scalar.copy(out=te, in_=thps[:, hs])
                t1 = stage.tile([P, 512], F32, tag="pt")
                r1 = stage.tile([P, 512], I32, tag="pr")
                u1 = stage.tile([P, 512], F32, tag="pu")
                red = stage.tile([P, 512], F32, tag="pred")
                nc.vector.tensor_scalar_mul(t1, te, 1.0 / TWOPI)
                nc.vector.tensor_copy(out=r1, in_=t1)
                nc.vector.tensor_scalar_mul(u1, r1, -TWOPI)
                nc.vector.tensor_add(red, te, u1)
                nc.scalar.activation(out=sj[:, hs], in_=red,
                                     func=mybir.ActivationFunctionType.Sin)
                t2 = stage.tile([P, 512], F32, tag="qt")
                r2 = stage.tile([P, 512], I32, tag="qr")
                u2 = stage.tile([P, 512], F32, tag="qu")
                red2 = stage.tile([P, 512], F32, tag="qred")
                nc.gpsimd.tensor_scalar(t2, te, 1.0 / TWOPI, 0.25,
                                        op0=mybir.AluOpType.mult,
                                        op1=mybir.AluOpType.add)
                nc.vector.tensor_copy(out=r2, in_=t2)
                nc.gpsimd.tensor_scalar(u2, r2, -TWOPI, math.pi / 2,
                                        op0=mybir.AluOpType.mult,
                                        op1=mybir.AluOpType.add)
                nc.gpsimd.tensor_add(red2, te, u2)
                nc.scalar.activation(out=cj[:, hs], in_=red2,
                                     func=mybir.ActivationFunctionType.Sin)
            sT.append(sj)
            cT.append(cj)
        ps0.__exit__(None, None, None)

        # ---- main loop ----
        pst_pool = ctx.enter_context(tc.tile_pool(name="pst", bufs=1,
                                                  space="PSUM"))
        psf = ctx.enter_context(tc.tile_pool(name="psf", bufs=3, space="PSUM"))
        psi = ctx.enter_context(tc.tile_pool(name="psi", bufs=2, space="PSUM"))
        for rep in range(reps):
          for b in range(BS):
            for h in range(2):
                n0 = h * ROWTILE
                # load 4 row groups; even/odd fold on Pool/DVE
                eog = []
                for g in range(NG):
                    t = xio.tile([P, D], F32, tag=f"x{g % 2}")
                    nc.sync.dma_start(
                        out=t, in_=x[b, n0 + g * P:n0 + (g + 1) * P, :])
                    eo = xio.tile([P, D], F32, tag=f"eo{g}")
                    nc.gpsimd.tensor_add(eo[:, 1:384], t[:, 1:384],
                                         t[:, 767:384:-1])
                    nc.gpsimd.tensor_sub(eo[:, 385:768], t[:, 1:384],
                                         t[:, 767:384:-1])
                    nc.vector.tensor_copy(out=eo[:, 0:385:384],
                                          in_=t[:, 0:385:384])
                    eog.append(eo)
                # transpose eo to (d', rows): 6 chunks of (128, 512), fp32r
                XT = []
                for c in range(NCH):
                    pst = pst_pool.tile([P, ROWTILE], F32, tag="pst")
                    for g in range(NG):
                        nc.tensor.transpose(pst[:, g * P:(g + 1) * P],
                                            eog[g][:, c * P:(c + 1) * P],
                                            ident)
                    xt = work.tile([P, ROWTILE], mm_dtype, tag=f"xt{c}")
                    nc.scalar.copy(out=xt, in_=pst)
                    XT.append(xt)
                # forward (block-sparse) + rotation per pair (j, 3+j)
                RI = [None] * NCH
                for j in range(3):
                    pR = psf.tile([P, ROWTILE], F32, tag="psf")
                    pI = psf.tile([P, ROWTILE], F32, tag="psf")
                    kR = FWD_BLOCKS[j]
                    for i, c in enumerate(kR):
                        nc.tensor.matmul(pR, FPt[c][:, j * P:(j + 1) * P],
                                         XT[c], start=(i == 0),
                                         stop=(i == len(kR) - 1))
                    kI = FWD_BLOCKS[3 + j]
                    for i, c in enumerate(kI):
                        nc.tensor.matmul(pI,
                                         FPt[c][:, (3 + j) * P:(4 + j) * P],
                                         XT[c], start=(i == 0),
                                         stop=(i == len(kI) - 1))
                    cs = cT[j][:, n0:n0 + ROWTILE]
                    sn = sT[j][:, n0:n0 + ROWTILE]
                    t1 = work.tile([P, ROWTILE], F32, tag="rta")
                    t2 = work.tile([P, ROWTILE], F32, tag="rtb")
                    t3 = work.tile([P, ROWTILE], F32, tag="rtc")
                    t4 = work.tile([P, ROWTILE], F32, tag="rtd")
                    nc.vector.tensor_mul(t1, pR, cs)
                    nc.vector.tensor_mul(t3, pR, sn)
                    nc.vector.tensor_mul(t2, pI, sn)
                    nc.vector.tensor_mul(t4, pI, cs)
                    rp = work.tile([P, ROWTILE], mm_dtype, tag=f"ri{j}")
                    ip = work.tile([P, ROWTILE], mm_dtype, tag=f"ri{3 + j}")
                    nc.gpsimd.tensor_sub(rp, t1, t2)
                    nc.gpsimd.tensor_add(ip, t3, t4)
                    RI[j] = rp
                    RI[3 + j] = ip
                # inverse (folded): u (385) and v (383) psum, un-fold to osb
                for g in range(NG):
                    pa = psi.tile([P, 386], F32, tag="pa")
                    pb = psi.tile([P, 384], F32, tag="pb")
                    gs = slice(g * P, (g + 1) * P)
                    for i, c in enumerate((0, 1, 2, 3)):
                        nc.tensor.matmul(pa, RI[c][:, gs], GPt[c][:, 0:386],
                                         start=(i == 0), stop=(i == 3))
                    for i, c in enumerate((3, 4, 5)):
                        nc.tensor.matmul(pb, RI[c][:, gs], GPt[c][:, 386:770],
                                         start=(i == 0), stop=(i == 2))
                    vb = work.tile([P, 384], F32, tag="rta")
                    ua = work.tile([P, 386], F32, tag="rtb")
                    nc.scalar.copy(out=vb, in_=pb)
                    nc.scalar.copy(out=ua, in_=pa)
                    osb = xio.tile([P, D], F32, tag=f"eo{g}")
                    nc.gpsimd.tensor_sub(osb[:, 1:384], ua[:, 1:384],
                                         vb[:, 0:383])
                    nc.gpsimd.tensor_add(osb[:, 385:768], ua[:, 383:0:-1],
                                         vb[:, 382::-1])
                    nc.vector.tensor_copy(out=osb[:, 0:385:384],
                                          in_=ua[:, 0:385:384])
                    nc.sync.dma_start(
                        out=out[b, n0 + g * P:n0 + (g + 1) * P, :], in_=osb)
    nc.finalize()
    return nc


_NC_CACHE = {}


def kernel(x, circ, positions):
    x = np.ascontiguousarray(x, dtype=np.float32)
    circ = np.ascontiguousarray(circ, dtype=np.float32)
    positions = np.ascontiguousarray(positions, dtype=np.int32)
    if "nc" not in _NC_CACHE:
        _NC_CACHE["nc"] = build_kernel()
    nc = _NC_CACHE["nc"]
    FP, GP = _dft_matrices()
    in_maps = []
    for core in range(NCORES):
        in_maps.append({
            "x": x[core * BS:(core + 1) * BS],
            "circ": circ,
            "positions": positions,
            "fp_c": FP,
            "gp_c": GP,
        })
    res = bass_utils.run_bass_kernel_spmd(nc, in_maps,
                                          core_ids=list(range(NCORES)))
    out = np.concatenate([res.results[c]["out"] for c in range(NCORES)],
                         axis=0)
    return out


if __name__ == "__main__":
    rng = np.random.default_rng(0)
    x = rng.standard_normal((B, N, D)).astype(np.float32)
    circ = (rng.standard_normal((2, D)) * 0.01).astype(np.float32)
    positions = rng.integers(0, 32, (N, 2)).astype(np.int32)
    out = kernel(x=x, circ=circ, positions=positions)
    print("out", out.shape, out.dtype)


# revision 14
# speedup vs baseline: 59812.9116x; 967.2062x over previous
"""Trainium2 Bass kernel for nn_CirculantSTRING.

Math: out[b,n,:] = irfft(exp(i*theta(n,:)) * rfft(x[b,n,:]), n=D)
where theta(n,f) = 2*(p0[n]*Im(rfft(circ0))[f] + p1[n]*Im(rfft(circ1))[f]).

Per core (data-parallel over batch, 4 batches/core), folded real-DFT:
  - even/odd fold: eo = [e_0..e_383 | x_384, o_1..o_383],
    e_d = x_d + x_{768-d}, o_d = x_d - x_{768-d} (halves forward matmul work)
  - forward matmul to fi = [R_0..R_383 | R_384, I_1..I_383]
    (block-sparse F2: 24 of 36 (128,128) blocks)
  - phase rotation with on-device cos/sin(theta) tables; theta(n,0)=0 makes
    the R_384 slot (chunk 3, partition 0) pass through untouched
  - inverse matmul to uv = [u_0..u_384 | v_1..v_383] (7 matmuls/row-group),
    un-fold out[d] = u_d - v_d, out[768-d] = u_d + v_d
Matmuls in float32r (1 cyc/row on PE at N>=256, ~11 mantissa bits); phase
path in fp32.
"""
import math
from contextlib import ExitStack

import numpy as np

import concourse.bacc as bacc
import concourse.tile as tile
from concourse import mybir
from concourse import bass_utils
from concourse.masks import make_identity

F32 = mybir.dt.float32
F32R = mybir.dt.float32r
I32 = mybir.dt.int32

B, N, D = 32, 1024, 768
NCORES = 8
BS = B // NCORES
P = 128
NCH = D // P              # 6
ROWTILE = 512
NG = ROWTILE // P         # 4

TWOPI = 2.0 * math.pi

# forward block list: M-chunk -> list of K-chunks
FWD_BLOCKS = {0: [0, 1, 2, 3], 1: [0, 1, 2, 3], 2: [0, 1, 2, 3],
              3: [0, 1, 2, 3, 4, 5], 4: [3, 4, 5], 5: [3, 4, 5]}


def _dft_matrices():
    """Folded forward F2 (eo -> fi) and inverse G2 (fi -> uv)."""
    dd = np.arange(384)
    ff = np.arange(384)
    fo = np.arange(1, 384)
    du = np.arange(385)
    F2 = np.zeros((D, D), np.float32)
    F2[0:384, 0:384] = np.cos(2 * np.pi * np.outer(dd, ff) / D)
    F2[0:384, 384] = (-1.0) ** dd
    F2[384, 0:384] = (-1.0) ** ff
    F2[384, 384] = 1.0
    F2[385:768, 385:768] = -np.sin(2 * np.pi * np.outer(fo, fo) / D)
    w = np.full(385, 2.0)
    w[0] = 1.0
    w[384] = 1.0
    G2 = np.zeros((D, 770), np.float32)
    G2[0:384, 0:385] = (w[0:384, None]
                        * np.cos(2 * np.pi * np.outer(ff, du) / D)) / D
    G2[384, 0:385] = ((-1.0) ** du) / D
    G2[385:768, 386:769] = (2.0 * np.sin(2 * np.pi * np.outer(fo, fo) / D)) / D
    return F2, G2


def build_kernel(mm_dtype=F32R, reps=1, trace_sim=False):
    nc = bacc.Bacc("TRN2", target_bir_lowering=False, debug=False,
                   num_devices=NCORES)
    x = nc.dram_tensor("x", [BS, N, D], F32, kind="ExternalInput").ap()
    circ = nc.dram_tensor("circ", [2, D], F32, kind="ExternalInput").ap()
    positions = nc.dram_tensor("positions", [N, 2], I32,
                               kind="ExternalInput").ap()
    fp_c = nc.dram_tensor("fp_c", [D, D], F32, kind="ExternalInput").ap()
    gp_c = nc.dram_tensor("gp_c", [D, 770], F32, kind="ExternalInput").ap()
    out = nc.dram_tensor("out", [BS, N, D], F32, kind="ExternalOutput").ap()

    with tile.TileContext(nc, trace_sim=trace_sim) as tc, ExitStack() as ctx:
        consts = ctx.enter_context(tc.tile_pool(name="consts", bufs=1))
        stage = ctx.enter_context(tc.tile_pool(name="stage", bufs=1))
        tabs = ctx.enter_context(tc.tile_pool(name="tabs", bufs=1))
        xio = ctx.enter_context(tc.tile_pool(name="xio", bufs=2))
        work = ctx.enter_context(tc.tile_pool(name="work", bufs=2))

        ident = consts.tile([P, P], F32, tag="ident")
        make_identity(nc, ident)

        ps0 = tc.tile_pool(name="ps0", bufs=1, space="PSUM")
        psum = ps0.__enter__()

        # ---- circ odd-fold (for s2, in fp32) ----
        circR = tabs.tile([2, D], F32, tag="circR")
        nc.sync.dma_start(out=circR, in_=circ)
        ocr = tabs.tile([2, 384], F32, tag="ocr")
        nc.vector.memset(ocr[:, 0:1], 0.0)
        nc.vector.tensor_sub(ocr[:, 1:384], circR[:, 1:384],
                             circR[:, 767:384:-1])
        occ = []  # (128, 2) fp32, o-fold of circ on chunk 3..5 partitions
        for i in range(3):
            poc = psum.tile([P, 2], F32, tag="pocc")
            nc.tensor.transpose(poc, ocr[:, i * P:(i + 1) * P], ident[0:2, 0:2])
            so = tabs.tile([P, 2], F32, tag=f"occ{i}")
            nc.scalar.copy(out=so, in_=poc)
            occ.append(so)

        # ---- load DFT matrices; s2 matmul on fp32 staging of F2 chunks 3..5
        s2ps = psum.tile([2, 384], F32, tag="s2ps")
        FPt, GPt = [], []
        for name, src in (("fp", fp_c), ("gp", gp_c)):
            for c in range(NCH):
                st = stage.tile([P, 770], F32, tag="mstage")
                nc.sync.dma_start(out=st[:, 0:src.shape[1]], in_=src[c * P:(c + 1) * P, :])
                if name == "fp" and c >= 3:
                    nc.tensor.matmul(s2ps[:, 1:384], occ[c - 3],
                                     st[:, 385:768],
                                     start=(c == 3), stop=(c == 5))
                wdt = D if name == "fp" else 770
                t = consts.tile([P, wdt], mm_dtype, tag=f"{name}{c}")
                nc.scalar.copy(out=t, in_=st[:, 0:wdt])
                (FPt if name == "fp" else GPt).append(t)
        s2 = tabs.tile([2, 384], F32, tag="s2")
        nc.vector.memset(s2[:, 0:1], 0.0)
        nc.vector.tensor_copy(out=s2[:, 1:384], in_=s2ps[:, 1:384])

        # ---- positions ----
        posT = tabs.tile([2, N], I32, tag="posT")
        nc.sync.dma_start(out=posT, in_=positions.rearrange("n k -> k n"))
        posTf = tabs.tile([2, N], F32, tag="posTf")
        nc.vector.tensor_scalar_mul(posTf, posT, 2.0)

        # ---- theta -> cos/sin tables (3 chunks of (128, N)) ----
        cT, sT = [], []
        for j in range(3):
            thps = psum.tile([P, N], F32, tag="thps")
            for h in range(2):
                nc.tensor.matmul(thps[:, h * 512:(h + 1) * 512],
                                 s2[:, j * P:(j + 1) * P],
                                 posTf[:, h * 512:(h + 1) * 512],
                                 start=True, stop=True)
            sj = tabs.tile([P, N], F32, tag=f"sT{j}")
            cj = tabs.tile([P, N], F32, tag=f"cT{j}")
            for hh in range(2):
                hs = slice(hh * 512, (hh + 1) * 512)
                te = stage.tile([P, 512], F32, tag="te")
                nc.scalar.copy(out=te, in_=thps[:, hs])
                t1 = stage.tile([P, 512], F32, tag="pt")
                r1 = stage.tile([P, 512], I32, tag="pr")
                u1 = stage.tile([P, 512], F32, tag="pu")
                red = stage.tile([P, 512], F32, tag="pred")
                nc.vector.tensor_scalar_mul(t1, te, 1.0 / TWOPI)
                nc.vector.tensor_copy(out=r1, in_=t1)
                nc.vector.tensor_scalar_mul(u1, r1, -TWOPI)
                nc.vector.tensor_add(red, te, u1)
                nc.scalar.activation(out=sj[:, hs], in_=red,
                                     func=mybir.ActivationFunctionType.Sin)
                t2 = stage.tile([P, 512], F32, tag="qt")
                r2 = stage.tile([P, 512], I32, tag="qr")
                u2 = stage.tile([P, 512], F32, tag="qu")
                red2 = stage.tile([P, 512], F32, tag="qred")
                nc.gpsimd.tensor_scalar(t2, te, 1.0 / TWOPI, 0.25,
                                        op0=mybir.AluOpType.mult,
                                        op1=mybir.AluOpType.add)
                nc.vector.tensor_copy(out=r2, in_=t2)
                nc.gpsimd.tensor_scalar(u2, r2, -TWOPI, math.pi / 2,
                                        op0=mybir.AluOpType.mult,
                                        op1=mybir.AluOpType.add)
                nc.gpsimd.tensor_add(red2, te, u2)
                nc.scalar.activation(out=cj[:, hs], in_=red2,
                                     func=mybir.ActivationFunctionType.Sin)
            sT.append(sj)
            cT.append(cj)
        ps0.__exit__(None, None, None)

        # ---- main loop ----
        pst_pool = ctx.enter_context(tc.tile_pool(name="pst", bufs=1,
                                                  space="PSUM"))
        psf = ctx.enter_context(tc.tile_pool(name="psf", bufs=4, space="PSUM"))
        psi = ctx.enter_context(tc.tile_pool(name="psi", bufs=1, space="PSUM"))
        for rep in range(reps):
          for b in range(BS):
            for h in range(2):
                n0 = h * ROWTILE
                # load 4 row groups; even/odd fold on Pool/DVE
                eog = []
                for g in range(NG):
                    t = xio.tile([P, D], F32, tag=f"x{g % 2}")
                    nc.sync.dma_start(
                        out=t, in_=x[b, n0 + g * P:n0 + (g + 1) * P, :])
                    eo = xio.tile([P, D], F32, tag=f"eo{g}")
                    nc.gpsimd.tensor_add(eo[:, 1:384], t[:, 1:384],
                                         t[:, 767:384:-1])
                    nc.gpsimd.tensor_sub(eo[:, 385:768], t[:, 1:384],
                                         t[:, 767:384:-1])
                    nc.vector.tensor_copy(out=eo[:, 0:385:384],
                                          in_=t[:, 0:385:384])
                    eog.append(eo)
                # transpose eo to (d', rows): 6 chunks of (128, 512), fp32r
                XT = []
                for c in range(NCH):
                    pst = pst_pool.tile([P, ROWTILE], F32, tag="pst")
                    for g in range(NG):
                        nc.tensor.transpose(pst[:, g * P:(g + 1) * P],
                                            eog[g][:, c * P:(c + 1) * P],
                                            ident)
                    xt = work.tile([P, ROWTILE], mm_dtype, tag=f"xt{c}")
                    nc.scalar.copy(out=xt, in_=pst)
                    XT.append(xt)
                # forward (block-sparse) + rotation per pair (j, 3+j)
                RI = [None] * NCH
                for j in range(3):
                    pR = psf.tile([P, ROWTILE], F32, tag="psf")
                    pI = psf.tile([P, ROWTILE], F32, tag="psf")
                    kR = FWD_BLOCKS[j]
                    for i, c in enumerate(kR):
                        nc.tensor.matmul(pR, FPt[c][:, j * P:(j + 1) * P],
                                         XT[c], start=(i == 0),
                                         stop=(i == len(kR) - 1))
                    kI = FWD_BLOCKS[3 + j]
                    for i, c in enumerate(kI):
                        nc.tensor.matmul(pI,
                                         FPt[c][:, (3 + j) * P:(4 + j) * P],
                                         XT[c], start=(i == 0),
                                         stop=(i == len(kI) - 1))
                    cs = cT[j][:, n0:n0 + ROWTILE]
                    sn = sT[j][:, n0:n0 + ROWTILE]
                    t1 = work.tile([P, ROWTILE], F32, tag="rta")
                    t2 = work.tile([P, ROWTILE], F32, tag="rtb")
                    t3 = work.tile([P, ROWTILE], F32, tag="rtc")
                    t4 = work.tile([P, ROWTILE], F32, tag="rtd")
                    nc.vector.tensor_mul(t1, pR, cs)
                    nc.vector.tensor_mul(t3, pR, sn)
                    nc.vector.tensor_mul(t2, pI, sn)
                    nc.vector.tensor_mul(t4, pI, cs)
                    rp = work.tile([P, ROWTILE], mm_dtype, tag=f"ri{j}")
                    ip = work.tile([P, ROWTILE], mm_dtype, tag=f"ri{3 + j}")
                    nc.gpsimd.tensor_sub(rp, t1, t2)
                    nc.gpsimd.tensor_add(ip, t3, t4)
                    RI[j] = rp
                    RI[3 + j] = ip
                # inverse (folded): u (385) and v (383) psum, un-fold to osb
                for g in range(NG):
                    pa = psi.tile([P, 386], F32, tag="pa")
                    pb = psi.tile([P, 384], F32, tag="pb")
                    gs = slice(g * P, (g + 1) * P)
                    for i, c in enumerate((0, 1, 2, 3)):
                        nc.tensor.matmul(pa, RI[c][:, gs], GPt[c][:, 0:386],
                                         start=(i == 0), stop=(i == 3))
                    for i, c in enumerate((3, 4, 5)):
                        nc.tensor.matmul(pb, RI[c][:, gs], GPt[c][:, 386:770],
                                         start=(i == 0), stop=(i == 2))
                    vb = work.tile([P, 384], F32, tag="rta")
                    ua = work.tile([P, 386], F32, tag="rtb")
                    nc.scalar.copy(out=vb, in_=pb)
                    nc.scalar.copy(out=ua, in_=pa)
                    osb = xio.tile([P, D], F32, tag=f"eo{g}")
                    nc.gpsimd.tensor_sub(osb[:, 1:384], ua[:, 1:384],
                                         vb[:, 0:383])
                    nc.gpsimd.tensor_add(osb[:, 385:768], ua[:, 383:0:-1],
                                         vb[:, 382::-1])
                    nc.vector.tensor_copy(out=osb[:, 0:385:384],
                                          in_=ua[:, 0:385:384])
                    nc.sync.dma_start(
                        out=out[b, n0 + g * P:n0 + (g + 1) * P, :], in_=osb)
    nc.finalize()
    return nc


_NC_CACHE = {}


def kernel(x, circ, positions):
    x = np.ascontiguousarray(x, dtype=np.float32)
    circ = np.ascontiguousarray(circ, dtype=np.float32)
    positions = np.ascontiguousarray(positions, dtype=np.int32)
    if "nc" not in _NC_CACHE:
        _NC_CACHE["nc"] = build_kernel()
    nc = _NC_CACHE["nc"]
    FP, GP = _dft_matrices()
    in_maps = []
    for core in range(NCORES):
        in_maps.append({
            "x": x[core * BS:(core + 1) * BS],
            "circ": circ,
            "positions": positions,
            "fp_c": FP,
            "gp_c": GP,
        })
    res = bass_utils.run_bass_kernel_spmd(nc, in_maps,
                                          core_ids=list(range(NCORES)))
    out = np.concatenate([res.results[c]["out"] for c in range(NCORES)],
                         axis=0)
    return out


if __name__ == "__main__":
    rng = np.random.default_rng(0)
    x = rng.standard_normal((B, N, D)).astype(np.float32)
    circ = (rng.standard_normal((2, D)) * 0.01).astype(np.float32)
    positions = rng.integers(0, 32, (N, 2)).astype(np.int32)
    out = kernel(x=x, circ=circ, positions=positions)
    print("out", out.shape, out.dtype)
